# revision 6
# baseline (speedup 1.0000x reference)
"""Causal attention (with faithful missing-head-transpose reshape bug) on 8 Trainium2 cores.

Problem: B=2, T=2048, E=1024, H=16, dk=64.
  qkv = x @ w_qkv.T ; q,k,v split; per-head causal softmax attention;
  out = att_out[B,H,T,dk].reshape(B,T,E)  (NO head transpose — faithful bug);
  y = out @ w_proj.T + b_proj

Key observation: because of the missing transpose, output rows
y[b, 128h : 128h+128, :] depend ONLY on head h.  Sharding (batch x head-group)
over 8 cores therefore needs NO collectives: core c handles batch c//4 and
heads 4*(c%4) .. 4*(c%4)+3, producing output rows [512g, 512g+512) of batch b.

Per-core kernel (all matmuls fp32r):
  - host supplies x^T [E,T] and pre-transposed weight slices
  - Q^T,K^T [256,T] and V [T,256] via fp32r matmuls
  - scores computed TRANSPOSED: S^T[j,i] (keys on partitions) so that
    exp(S^T) is directly the lhsT-ready P^T for the P@V matmul
  - V is stored with a ones-column per head: the P@V matmul's row 64 is then
    the softmax denominator for free
  - causal masking via gpsimd.affine_select on the exp tiles (diag blocks only;
    fully-masked blocks are skipped entirely)
  - normalization: DVE reciprocal of denom row, broadcast across partitions
    with a K=1 outer-product matmul, then one DVE multiply
  - the buggy reshape is free in row-major DRAM; the projection's lhsT
    (R^T chunks) is read from a [dup, shift-by-one] att2 buffer with a
    stride-16 access pattern
"""

import os
import sys

import numpy as np

for _p in ("/opt/trn_rl_repo", "/root/.axon_site/_ro/trn_rl_repo"):
    if os.path.isdir(_p) and _p not in sys.path:
        sys.path.insert(0, _p)

import concourse.bass as bass  # noqa: E402
import concourse.bacc as bacc  # noqa: E402
import concourse.mybir as mybir  # noqa: E402
from concourse.bass import ds, ts  # noqa: E402
from concourse.tile import TileContext  # noqa: E402

F32 = mybir.dt.float32
F32R = mybir.dt.float32r
AF = mybir.ActivationFunctionType

P = 128
E = 1024
DK = 64
HPC = 4  # heads per core
TW = 512  # i-window for scores / pv matmuls
EC = E // P  # 8 e-chunks
DC = (HPC * DK) // P  # 2 chunks of per-core qk features
FW = E // 512  # 2 output-feature windows


def build_nc(T=2048):
    W = T // TW  # i-windows
    JPW = TW // P  # j-chunks per window (4)
    TC = T // P  # t-chunks for V
    RR = (T * DK) // E  # rows of R per head (T/16)
    TT = E // DK  # 16 t-positions per R row

    nc = bacc.Bacc("TRN2", target_bir_lowering=False, debug=False)
    xT = nc.declare_dram_parameter("xT", [E, T], F32R, isOutput=False)
    wqT = nc.declare_dram_parameter("wqT", [E, HPC * DK], F32R, isOutput=False)
    wkT = nc.declare_dram_parameter("wkT", [E, HPC * DK], F32R, isOutput=False)
    wvT = nc.declare_dram_parameter("wvT", [E, HPC * DK], F32R, isOutput=False)
    wpT = nc.declare_dram_parameter("wpT", [E, E], F32R, isOutput=False)
    bp = nc.declare_dram_parameter("bp", [1, E], F32R, isOutput=False)
    y = nc.declare_dram_parameter("y", [HPC * RR, E], F32, isOutput=True)

    with nc.allow_low_precision(reason="fp32r pipeline; softmax math is fp32 in PSUM"), TileContext(nc) as tc:
        with (
            tc.tile_pool(name="const", bufs=1) as const,
            tc.tile_pool(name="qkvout", bufs=1) as qkv_pool,
            tc.tile_pool(name="wp", bufs=1) as wp_pool,
        ):
            ones = const.tile([P, P], F32R)
            nc.vector.memset(ones.bitcast(F32), 1.0)
            bp_sb = const.tile([1, E], F32R)
            nc.sync.dma_start(out=bp_sb, in_=bp[:, :])

            wp_sb = wp_pool.tile([P, EC, E], F32R)
            nc.sync.dma_start(out=wp_sb, in_=wpT[:, :].rearrange("(e p) f -> p e f", p=P))

            qT = qkv_pool.tile([P, DC, T], F32R)
            kT = qkv_pool.tile([P, DC, T], F32R)
            vsb = qkv_pool.tile([P, TC, HPC * (DK + 1)], F32R)
            # ones column per head (row 64 of each P@V psum = softmax denominator)
            nc.vector.memset(
                vsb.rearrange("p t (h c) -> p t h c", c=DK + 1)[:, :, :, DK : DK + 1].bitcast(F32),
                1.0,
            )

            # ---------------- QKV projections ----------------
            with (
                tc.tile_pool(name="xin", bufs=1) as xpool,
                tc.tile_pool(name="wqkv", bufs=1) as wq_pool,
                tc.tile_pool(name="psq", bufs=4, space="PSUM") as psq,
            ):
                wq_sb = wq_pool.tile([P, EC, HPC * DK], F32R)
                wk_sb = wq_pool.tile([P, EC, HPC * DK], F32R)
                wv_sb = wq_pool.tile([P, EC, HPC * DK], F32R)
                nc.sync.dma_start(out=wq_sb, in_=wqT[:, :].rearrange("(e p) d -> p e d", p=P))
                nc.sync.dma_start(out=wk_sb, in_=wkT[:, :].rearrange("(e p) d -> p e d", p=P))
                nc.sync.dma_start(out=wv_sb, in_=wvT[:, :].rearrange("(e p) d -> p e d", p=P))

                xp = xpool.tile([P, EC, T], F32R)
                for e in range(EC):
                    nc.sync.dma_start(out=xp[:, e, :], in_=xT[ts(e, P), :])

                # Q^T, K^T : [dloc (part), T]
                for dst, wsb in ((qT, wq_sb), (kT, wk_sb)):
                    for dc in range(DC):
                        for w in range(T // TW):
                            ps = psq.tile([P, TW], F32, tag="qa", name="ps_qk")
                            for e in range(EC):
                                nc.tensor.matmul(
                                    ps,
                                    wsb[:, e, ts(dc, P)],
                                    xp[:, e, ds(TW * w, TW)],
                                    start=(e == 0),
                                    stop=(e == EC - 1),
                                )
                            nc.vector.tensor_copy(dst[:, dc, ds(TW * w, TW)], ps)

                # V natural : [t (part), d]
                for t in range(TC):
                    psv = psq.tile([P, HPC * DK], F32, tag="qa", name="ps_v")
                    for e in range(EC):
                        nc.tensor.matmul(
                            psv,
                            xp[:, e, ts(t, P)],
                            wv_sb[:, e, :],
                            start=(e == 0),
                            stop=(e == EC - 1),
                        )
                    nc.vector.tensor_copy(
                        vsb.rearrange("p t (h c) -> p t h c", c=DK + 1)[:, t, :, 0:DK],
                        psv.rearrange("p (h d) -> p h d", d=DK),
                    )

            # ---------------- attention ----------------
            with tc.tile_pool(name="att", bufs=1) as att_pool:
                att2 = []
                for h in range(HPC):
                    a = att_pool.tile([P, T], F32R, name=f"att2_{h}", tag=f"att2_{h}")
                    att2.append(a)
                    # last col of shifted half never written; keep sim happy
                    nc.vector.memset(a[DK : 2 * DK, T - 1 : T].bitcast(F32), 0.0)

                with (
                    tc.tile_pool(name="exps", bufs=6) as epool,
                    tc.tile_pool(name="rec", bufs=2) as rpool,
                    tc.tile_pool(name="psa", bufs=1, space="PSUM") as psa,
                ):
                    for w in range(W):
                        pvs = [
                            psa.tile([P, TW], F32, tag=f"pv{h}", bufs=1, name=f"pv{h}")
                            for h in range(HPC)
                        ]
                        njc = JPW * (w + 1)
                        for jc in range(njc):
                            qq = jc - JPW * w  # >=0 on causal-diagonal blocks
                            for h in range(HPC):
                                dc, sub = h // 2, h % 2
                                s = psa.tile([P, TW], F32, tag="s", bufs=3, name="s")
                                nc.tensor.matmul(
                                    s,
                                    kT[ds(DK * sub, DK), dc, ts(jc, P)],
                                    qT[ds(DK * sub, DK), dc, ds(TW * w, TW)],
                                    start=True,
                                    stop=True,
                                )
                                es = epool.tile([P, TW], F32R, name="es")
                                nc.scalar.activation(es, s, AF.Exp, scale=1.0 / 8.0)
                                if qq >= 0:
                                    # keep where i - j - 128*qq >= 0 else 0
                                    nc.gpsimd.affine_select(
                                        out=es,
                                        in_=es,
                                        pattern=[[1, TW]],
                                        compare_op=mybir.AluOpType.is_ge,
                                        fill=0.0,
                                        base=-P * qq,
                                        channel_multiplier=-1,
                                    )
                                nc.tensor.matmul(
                                    pvs[h][0 : DK + 1, :],
                                    vsb[:, jc, ds((DK + 1) * h, DK + 1)],
                                    es,
                                    start=(jc == 0),
                                    stop=(jc == njc - 1),
                                )
                        for h in range(HPC):
                            rec = rpool.tile([P, TW], F32R, name="rec")
                            nc.vector.reciprocal(rec[DK : DK + 1, :], pvs[h][DK : DK + 1, :])
                            rt = psa.tile([P, TW], F32, tag="rt", bufs=1, name="rt")
                            nc.tensor.matmul(
                                rt[0:DK, :],
                                ones[DK : DK + 1, 0:DK],
                                rec[DK : DK + 1, :],
                                start=True,
                                stop=True,
                            )
                            # DVE cannot read two PSUM operands; stage recip bcast in SBUF
                            nc.vector.tensor_copy(rec[0:DK, :], rt[0:DK, :])
                            nc.vector.tensor_mul(
                                att2[h][0:DK, ds(TW * w, TW)], pvs[h][0:DK, :], rec[0:DK, :]
                            )
                            # shifted duplicate for the odd-tt half of R^T
                            if w == 0:
                                nc.sync.dma_start(
                                    out=att2[h][DK : 2 * DK, 0 : TW - 1],
                                    in_=att2[h][0:DK, 1:TW],
                                )
                            else:
                                nc.sync.dma_start(
                                    out=att2[h][DK : 2 * DK, TW * w - 1 : TW * (w + 1) - 1],
                                    in_=att2[h][0:DK, ds(TW * w, TW)],
                                )

                # ---------------- output projection ----------------
                with (
                    tc.tile_pool(name="yout", bufs=2) as ypool,
                    tc.tile_pool(name="psy", bufs=2, space="PSUM") as psy,
                ):
                    for h in range(HPC):
                        a2v = att2[h].rearrange("p (r t) -> p r t", t=TT)  # [P, RR, TT]
                        for fw in range(FW):
                            yp = psy.tile([P, 512], F32, tag="y", name="yp")
                            for m in range(EC):
                                nc.tensor.matmul(
                                    yp[0:RR, :],
                                    a2v[:, :, 2 * m : 2 * m + 1],
                                    wp_sb[:, m, ds(512 * fw, 512)],
                                    start=(m == 0),
                                    stop=False,
                                )
                            nc.tensor.matmul(
                                yp[0:RR, :],
                                ones[0:1, 0:RR],
                                bp_sb[0:1, ds(512 * fw, 512)],
                                start=False,
                                stop=True,
                            )
                            ysb = ypool.tile([P, 512], F32, name="ysb")
                            nc.vector.tensor_copy(ysb[0:RR, :], yp[0:RR, :])
                            nc.sync.dma_start(
                                out=y[ds(RR * h, RR), ds(512 * fw, 512)], in_=ysb[0:RR, :]
                            )
    nc.compile()
    return nc


_CACHE = {}
LAST_RESULT = None


def _get_nc(T=2048):
    key = ("nc", T)
    if key not in _CACHE:
        _CACHE[key] = build_nc(T=T)
    return _CACHE[key]


def make_in_maps(x, w_qkv, w_proj, b_proj):
    B, T, _E = x.shape
    in_maps = []
    wpTh = np.ascontiguousarray(w_proj.T.astype(np.float32))
    bph = np.ascontiguousarray(b_proj.reshape(1, E).astype(np.float32))
    xTs = [np.ascontiguousarray(x[b].T.astype(np.float32)) for b in range(B)]
    for c in range(8):
        b, g = divmod(c, 4)
        r0 = HPC * DK * g  # 256*g
        sl = slice(r0, r0 + HPC * DK)
        in_maps.append(
            {
                "xT": xTs[b],
                "wqT": np.ascontiguousarray(w_qkv[sl, :].T.astype(np.float32)),
                "wkT": np.ascontiguousarray(w_qkv[E:][sl, :].T.astype(np.float32)),
                "wvT": np.ascontiguousarray(w_qkv[2 * E :][sl, :].T.astype(np.float32)),
                "wpT": wpTh,
                "bp": bph,
            }
        )
    return in_maps


def kernel(x, w_qkv, w_proj, b_proj):
    global LAST_RESULT
    from concourse.bass_utils import run_bass_kernel_spmd

    x = np.asarray(x, dtype=np.float32)
    w_qkv = np.asarray(w_qkv, dtype=np.float32)
    w_proj = np.asarray(w_proj, dtype=np.float32)
    b_proj = np.asarray(b_proj, dtype=np.float32)
    B, T, _E = x.shape

    nc = _get_nc(T=T)
    in_maps = make_in_maps(x, w_qkv, w_proj, b_proj)
    res = run_bass_kernel_spmd(nc, in_maps, core_ids=list(range(8)))
    LAST_RESULT = res

    out = np.empty((B, T, E), dtype=np.float32)
    rows = HPC * ((T * DK) // E)  # 512 rows per core
    for c in range(8):
        b, g = divmod(c, 4)
        out[b, rows * g : rows * (g + 1), :] = res.results[c]["y"]
    return out


# revision 10
# speedup vs baseline: 1.2285x; 1.2285x over previous
"""Causal attention (with faithful missing-head-transpose reshape bug) on 8 Trainium2 cores.

Problem: B=2, T=2048, E=1024, H=16, dk=64.
  qkv = x @ w_qkv.T ; q,k,v split; per-head causal softmax attention;
  out = att_out[B,H,T,dk].reshape(B,T,E)  (NO head transpose — faithful bug);
  y = out @ w_proj.T + b_proj

Key observation: because of the missing transpose, output rows
y[b, 128h : 128h+128, :] depend ONLY on head h.  Sharding (batch x head-group)
over 8 cores therefore needs NO collectives: core c handles batch c//4 and
heads 4*(c%4) .. 4*(c%4)+3, producing output rows [512g, 512g+512) of batch b.

Per-core kernel (bf16 matmuls, fp32 PSUM accumulation, fp32 softmax math):
  - host supplies x^T [E,T] and pre-transposed weight slices in bf16
  - Q^T,K^T [256,T] and V [T,256] via matmuls
  - scores computed TRANSPOSED: S^T[j,i] (keys on partitions) so that
    exp(S^T) is directly the lhsT-ready P^T for the P@V matmul
  - V is stored with a ones-column per head: the P@V matmul's row 64 is then
    the softmax denominator for free
  - causal masking via gpsimd.affine_select on the exp tiles (diag blocks only;
    fully-masked blocks are skipped entirely)
  - normalization: fast-approx reciprocal of the denom row, broadcast across
    partitions with a K=1 outer-product matmul, one DVE multiply
  - the buggy reshape is free in row-major DRAM; the projection's lhsT
    (R^T chunks) is read from a [dup, shift-by-one] att2 buffer with a
    stride-16 access pattern
"""

import os
import sys

import numpy as np

for _p in ("/opt/trn_rl_repo", "/root/.axon_site/_ro/trn_rl_repo"):
    if os.path.isdir(_p) and _p not in sys.path:
        sys.path.insert(0, _p)

import ml_dtypes  # noqa: E402

import concourse.bacc as bacc  # noqa: E402
import concourse.mybir as mybir  # noqa: E402
from concourse.bass import ds, ts  # noqa: E402
from concourse.tile import TileContext  # noqa: E402

F32 = mybir.dt.float32
BF16 = mybir.dt.bfloat16
AF = mybir.ActivationFunctionType
BF16NP = ml_dtypes.bfloat16

P = 128
E = 1024
DK = 64
HPC = 4  # heads per core
TW = 512  # i-window for scores / pv matmuls
EC = E // P  # 8 e-chunks
DC = (HPC * DK) // P  # 2 chunks of per-core qk features
FW = E // 512  # 2 output-feature windows


def build_nc(T=2048):
    W = T // TW  # i-windows
    JPW = TW // P  # j-chunks per window (4)
    TC = T // P  # t-chunks for V
    RR = (T * DK) // E  # rows of R per head (T/16)
    TT = E // DK  # 16 t-positions per R row

    nc = bacc.Bacc("TRN2", target_bir_lowering=False, debug=False)
    xT = nc.declare_dram_parameter("xT", [E, T], BF16, isOutput=False)
    wqT = nc.declare_dram_parameter("wqT", [E, HPC * DK], BF16, isOutput=False)
    wkT = nc.declare_dram_parameter("wkT", [E, HPC * DK], BF16, isOutput=False)
    wvT = nc.declare_dram_parameter("wvT", [E, HPC * DK], BF16, isOutput=False)
    wpT = nc.declare_dram_parameter("wpT", [E, E], BF16, isOutput=False)
    bp = nc.declare_dram_parameter("bp", [1, E], BF16, isOutput=False)
    y = nc.declare_dram_parameter("y", [HPC * RR, E], F32, isOutput=True)

    with nc.allow_low_precision(reason="bf16 matmuls; accumulation stays fp32 in PSUM"), TileContext(nc) as tc:
        with (
            tc.tile_pool(name="const", bufs=1) as const,
            tc.tile_pool(name="qkvout", bufs=1) as qkv_pool,
            tc.tile_pool(name="wp", bufs=1) as wp_pool,
        ):
            ones = const.tile([P, P], BF16)
            nc.vector.memset(ones, 1.0)
            bp_sb = const.tile([1, E], BF16)
            nc.sync.dma_start(out=bp_sb, in_=bp[:, :])

            wp_sb = wp_pool.tile([P, EC, E], BF16)
            nc.sync.dma_start(out=wp_sb, in_=wpT[:, :].rearrange("(e p) f -> p e f", p=P))

            qT = qkv_pool.tile([P, DC, T], BF16)
            kT = qkv_pool.tile([P, DC, T], BF16)
            vsb = qkv_pool.tile([P, TC, HPC * (DK + 1)], BF16)
            # ones column per head (row 64 of each P@V psum = softmax denominator)
            nc.vector.memset(
                vsb.rearrange("p t (h c) -> p t h c", c=DK + 1)[:, :, :, DK : DK + 1], 1.0
            )

            # ---------------- QKV projections ----------------
            with (
                tc.tile_pool(name="xin", bufs=1) as xpool,
                tc.tile_pool(name="wqkv", bufs=1) as wq_pool,
                tc.tile_pool(name="psq", bufs=4, space="PSUM") as psq,
            ):
                wq_sb = wq_pool.tile([P, EC, HPC * DK], BF16)
                wk_sb = wq_pool.tile([P, EC, HPC * DK], BF16)
                wv_sb = wq_pool.tile([P, EC, HPC * DK], BF16)
                nc.sync.dma_start(out=wq_sb, in_=wqT[:, :].rearrange("(e p) d -> p e d", p=P))
                nc.sync.dma_start(out=wk_sb, in_=wkT[:, :].rearrange("(e p) d -> p e d", p=P))
                nc.sync.dma_start(out=wv_sb, in_=wvT[:, :].rearrange("(e p) d -> p e d", p=P))

                xp = xpool.tile([P, EC, T], BF16)
                for e in range(EC):
                    nc.sync.dma_start(out=xp[:, e, :], in_=xT[ts(e, P), :])

                # Q^T, K^T : [dloc (part), T]
                for dst, wsb in ((qT, wq_sb), (kT, wk_sb)):
                    for dc in range(DC):
                        for w in range(T // TW):
                            ps = psq.tile([P, TW], F32, tag="qa", name="ps_qk")
                            for e in range(EC):
                                nc.tensor.matmul(
                                    ps,
                                    wsb[:, e, ts(dc, P)],
                                    xp[:, e, ds(TW * w, TW)],
                                    start=(e == 0),
                                    stop=(e == EC - 1),
                                )
                            nc.vector.tensor_copy(dst[:, dc, ds(TW * w, TW)], ps)

                # V natural : [t (part), d]
                for t in range(TC):
                    psv = psq.tile([P, HPC * DK], F32, tag="qa", name="ps_v")
                    for e in range(EC):
                        nc.tensor.matmul(
                            psv,
                            xp[:, e, ts(t, P)],
                            wv_sb[:, e, :],
                            start=(e == 0),
                            stop=(e == EC - 1),
                        )
                    nc.vector.tensor_copy(
                        vsb.rearrange("p t (h c) -> p t h c", c=DK + 1)[:, t, :, 0:DK],
                        psv.rearrange("p (h d) -> p h d", d=DK),
                    )

            # ---------------- attention ----------------
            with tc.tile_pool(name="att", bufs=1) as att_pool:
                att2 = []
                for h in range(HPC):
                    a = att_pool.tile([P, T], BF16, name=f"att2_{h}", tag=f"att2_{h}")
                    att2.append(a)
                    # last col of shifted half never written; keep sim happy
                    nc.vector.memset(a[DK : 2 * DK, T - 1 : T], 0.0)

                with (
                    tc.tile_pool(name="exps", bufs=8) as epool,
                    tc.tile_pool(name="rec", bufs=4) as rpool,
                    tc.tile_pool(name="psa", bufs=1, space="PSUM") as psa,
                ):
                    for w in range(W):
                        pvs = [
                            psa.tile([P, TW], F32, tag=f"pv{h}", bufs=1, name=f"pv{h}")
                            for h in range(HPC)
                        ]
                        njc = JPW * (w + 1)
                        for jc in range(njc):
                            qq = jc - JPW * w  # >=0 on causal-diagonal blocks
                            for h in range(HPC):
                                dc, sub = h // 2, h % 2
                                s = psa.tile([P, TW], F32, tag="s", bufs=3, name="s")
                                nc.tensor.matmul(
                                    s,
                                    kT[ds(DK * sub, DK), dc, ts(jc, P)],
                                    qT[ds(DK * sub, DK), dc, ds(TW * w, TW)],
                                    start=True,
                                    stop=True,
                                )
                                es = epool.tile([P, TW], BF16, name="es")
                                nc.scalar.activation(es, s, AF.Exp, scale=1.0 / 8.0)
                                if qq >= 0:
                                    # keep where i - j - 128*qq >= 0 else 0
                                    nc.gpsimd.affine_select(
                                        out=es,
                                        in_=es,
                                        pattern=[[1, TW]],
                                        compare_op=mybir.AluOpType.is_ge,
                                        fill=0.0,
                                        base=-P * qq,
                                        channel_multiplier=-1,
                                    )
                                nc.tensor.matmul(
                                    pvs[h][0 : DK + 1, :],
                                    vsb[:, jc, ds((DK + 1) * h, DK + 1)],
                                    es,
                                    start=(jc == 0),
                                    stop=(jc == njc - 1),
                                )
                        for h in range(HPC):
                            # drain pv psum quickly: raw attention rows + denom row
                            praw = rpool.tile([P, TW], BF16, name="praw", tag="praw")
                            nc.vector.tensor_copy(praw[0:DK, :], pvs[h][0:DK, :])
                            dn = rpool.tile([P, TW], F32, name="dn", tag="dn")
                            nc.vector.tensor_copy(dn[DK : DK + 1, :], pvs[h][DK : DK + 1, :])
                            # 1/denom (fast approx, ~18 bits) then bf16 for the bcast matmul
                            recf = rpool.tile([P, TW], F32, name="recf", tag="recf")
                            nc.vector.reciprocal(
                                out=recf[DK : DK + 1, :], in_=dn[DK : DK + 1, :]
                            )
                            recb = rpool.tile([P, TW], BF16, name="recb", tag="recb")
                            nc.vector.tensor_copy(recb[DK : DK + 1, :], recf[DK : DK + 1, :])
                            rt = psa.tile([P, TW], F32, tag="rt", bufs=1, name="rt")
                            nc.tensor.matmul(
                                rt[0:DK, :],
                                ones[DK : DK + 1, 0:DK],
                                recb[DK : DK + 1, :],
                                start=True,
                                stop=True,
                            )
                            nc.vector.tensor_mul(
                                att2[h][0:DK, ds(TW * w, TW)], rt[0:DK, :], praw[0:DK, :]
                            )
                            # shifted duplicate for the odd-tt half of R^T
                            if w == 0:
                                nc.sync.dma_start(
                                    out=att2[h][DK : 2 * DK, 0 : TW - 1],
                                    in_=att2[h][0:DK, 1:TW],
                                )
                            else:
                                nc.sync.dma_start(
                                    out=att2[h][DK : 2 * DK, TW * w - 1 : TW * (w + 1) - 1],
                                    in_=att2[h][0:DK, ds(TW * w, TW)],
                                )

                # ---------------- output projection ----------------
                with (
                    tc.tile_pool(name="yout", bufs=2) as ypool,
                    tc.tile_pool(name="psy", bufs=2, space="PSUM") as psy,
                ):
                    for h in range(HPC):
                        a2v = att2[h].rearrange("p (r t) -> p r t", t=TT)  # [P, RR, TT]
                        for fw in range(FW):
                            yp = psy.tile([P, 512], F32, tag="y", name="yp")
                            for m in range(EC):
                                nc.tensor.matmul(
                                    yp[0:RR, :],
                                    a2v[:, :, 2 * m : 2 * m + 1],
                                    wp_sb[:, m, ds(512 * fw, 512)],
                                    start=(m == 0),
                                    stop=False,
                                )
                            nc.tensor.matmul(
                                yp[0:RR, :],
                                ones[0:1, 0:RR],
                                bp_sb[0:1, ds(512 * fw, 512)],
                                start=False,
                                stop=True,
                            )
                            ysb = ypool.tile([P, 512], F32, name="ysb")
                            nc.vector.tensor_copy(ysb[0:RR, :], yp[0:RR, :])
                            nc.sync.dma_start(
                                out=y[ds(RR * h, RR), ds(512 * fw, 512)], in_=ysb[0:RR, :]
                            )
    nc.compile()
    return nc


_CACHE = {}
LAST_RESULT = None


def _get_nc(T=2048):
    key = ("nc", T)
    if key not in _CACHE:
        _CACHE[key] = build_nc(T=T)
    return _CACHE[key]


def make_in_maps(x, w_qkv, w_proj, b_proj):
    B, T, _E = x.shape
    in_maps = []
    wpTh = np.ascontiguousarray(w_proj.T.astype(BF16NP))
    bph = np.ascontiguousarray(b_proj.reshape(1, E).astype(BF16NP))
    xTs = [np.ascontiguousarray(x[b].T.astype(BF16NP)) for b in range(B)]
    for c in range(8):
        b, g = divmod(c, 4)
        r0 = HPC * DK * g  # 256*g
        sl = slice(r0, r0 + HPC * DK)
        in_maps.append(
            {
                "xT": xTs[b],
                "wqT": np.ascontiguousarray(w_qkv[sl, :].T.astype(BF16NP)),
                "wkT": np.ascontiguousarray(w_qkv[E:][sl, :].T.astype(BF16NP)),
                "wvT": np.ascontiguousarray(w_qkv[2 * E :][sl, :].T.astype(BF16NP)),
                "wpT": wpTh,
                "bp": bph,
            }
        )
    return in_maps


def kernel(x, w_qkv, w_proj, b_proj):
    global LAST_RESULT
    from concourse.bass_utils import run_bass_kernel_spmd

    x = np.asarray(x, dtype=np.float32)
    w_qkv = np.asarray(w_qkv, dtype=np.float32)
    w_proj = np.asarray(w_proj, dtype=np.float32)
    b_proj = np.asarray(b_proj, dtype=np.float32)
    B, T, _E = x.shape

    nc = _get_nc(T=T)
    in_maps = make_in_maps(x, w_qkv, w_proj, b_proj)
    res = run_bass_kernel_spmd(nc, in_maps, core_ids=list(range(8)))
    LAST_RESULT = res

    out = np.empty((B, T, E), dtype=np.float32)
    rows = HPC * ((T * DK) // E)  # 512 rows per core
    for c in range(8):
        b, g = divmod(c, 4)
        out[b, rows * g : rows * (g + 1), :] = res.results[c]["y"]
    return out


# revision 13
# speedup vs baseline: 1.4556x; 1.1848x over previous
"""Causal attention (with faithful missing-head-transpose reshape bug) on 8 Trainium2 cores.

Problem: B=2, T=2048, E=1024, H=16, dk=64.
  qkv = x @ w_qkv.T ; q,k,v split; per-head causal softmax attention;
  out = att_out[B,H,T,dk].reshape(B,T,E)  (NO head transpose — faithful bug);
  y = out @ w_proj.T + b_proj

Key observation: because of the missing transpose, output rows
y[b, 128h : 128h+128, :] depend ONLY on head h.  Sharding (batch x head-group)
over 8 cores therefore needs NO collectives: core c handles batch c//4 and
heads 4*(c%4) .. 4*(c%4)+3, producing output rows [512g, 512g+512) of batch b.

Per-core kernel (bf16 matmuls, fp32 PSUM accumulation, fp32 softmax math):
  - host supplies x^T [E,T] and pre-transposed weight slices in bf16
  - Q^T,K^T [256,T] and V [T,256] via matmuls
  - scores computed TRANSPOSED: S^T[j,i] (keys on partitions) so that
    exp(S^T) is directly the lhsT-ready P^T for the P@V matmul
  - V is stored with a ones-column per head: the P@V matmul's row 64 is then
    the softmax denominator for free
  - causal masking via gpsimd.affine_select on the exp tiles (diag blocks only;
    fully-masked blocks are skipped entirely)
  - normalization: fast-approx reciprocal of the denom row, broadcast across
    partitions with a K=1 outer-product matmul, one DVE multiply
  - the buggy reshape is free in row-major DRAM; the projection's lhsT
    (R^T chunks) is read from a [dup, shift-by-one] att2 buffer with a
    stride-16 access pattern
"""

import os
import sys

import numpy as np

for _p in ("/opt/trn_rl_repo", "/root/.axon_site/_ro/trn_rl_repo"):
    if os.path.isdir(_p) and _p not in sys.path:
        sys.path.insert(0, _p)

import ml_dtypes  # noqa: E402

import concourse.bacc as bacc  # noqa: E402
import concourse.mybir as mybir  # noqa: E402
from concourse.bass import ds, ts  # noqa: E402
from concourse.tile import TileContext  # noqa: E402

F32 = mybir.dt.float32
BF16 = mybir.dt.bfloat16
AF = mybir.ActivationFunctionType
BF16NP = ml_dtypes.bfloat16

P = 128
E = 1024
DK = 64
HPC = 4  # heads per core
TW = 512  # i-window for scores / pv matmuls
EC = E // P  # 8 e-chunks
DC = (HPC * DK) // P  # 2 chunks of per-core qk features
FW = E // 512  # 2 output-feature windows


def build_nc(T=2048):
    W = T // TW  # i-windows
    JPW = TW // P  # j-chunks per window (4)
    TC = T // P  # t-chunks for V
    RR = (T * DK) // E  # rows of R per head (T/16)
    TT = E // DK  # 16 t-positions per R row

    nc = bacc.Bacc("TRN2", target_bir_lowering=False, debug=False)
    xT = nc.declare_dram_parameter("xT", [E, T], BF16, isOutput=False)
    wqT = nc.declare_dram_parameter("wqT", [E, HPC * DK], BF16, isOutput=False)
    wkT = nc.declare_dram_parameter("wkT", [E, HPC * DK], BF16, isOutput=False)
    wvT = nc.declare_dram_parameter("wvT", [E, HPC * DK], BF16, isOutput=False)
    wpT = nc.declare_dram_parameter("wpT", [E, E], BF16, isOutput=False)
    bp = nc.declare_dram_parameter("bp", [1, E], BF16, isOutput=False)
    y = nc.declare_dram_parameter("y", [HPC * RR, E], F32, isOutput=True)

    with nc.allow_low_precision(reason="bf16 matmuls; accumulation stays fp32 in PSUM"), TileContext(nc) as tc:
        with (
            tc.tile_pool(name="const", bufs=1) as const,
            tc.tile_pool(name="qkvout", bufs=1) as qkv_pool,
            tc.tile_pool(name="wp", bufs=1) as wp_pool,
        ):
            ones = const.tile([P, P], BF16)
            nc.vector.memset(ones, 1.0)
            bp_sb = const.tile([1, E], BF16)
            nc.sync.dma_start(out=bp_sb, in_=bp[:, :])

            wp_sb = wp_pool.tile([P, EC, E], BF16)
            nc.sync.dma_start(out=wp_sb, in_=wpT[:, :].rearrange("(e p) f -> p e f", p=P))

            qT = qkv_pool.tile([P, DC, T], BF16)
            kT = qkv_pool.tile([P, DC, T], BF16)
            vsb = qkv_pool.tile([P, TC, HPC * (DK + 1)], BF16)
            # ones column per head (row 64 of each P@V psum = softmax denominator)
            nc.vector.memset(
                vsb.rearrange("p t (h c) -> p t h c", c=DK + 1)[:, :, :, DK : DK + 1], 1.0
            )
            zer = const.tile([P, DK + 1], BF16)
            nc.vector.memset(zer, 0.0)
            wsrc = const.tile([P, TW], BF16)
            nc.vector.memset(wsrc, 0.0)

            # ---------------- QKV projections ----------------
            with (
                tc.tile_pool(name="xin", bufs=1) as xpool,
                tc.tile_pool(name="wqkv", bufs=1) as wq_pool,
                tc.tile_pool(name="psq", bufs=4, space="PSUM") as psq,
            ):
                wq_sb = wq_pool.tile([P, EC, HPC * DK], BF16)
                wk_sb = wq_pool.tile([P, EC, HPC * DK], BF16)
                wv_sb = wq_pool.tile([P, EC, HPC * DK], BF16)
                nc.sync.dma_start(out=wq_sb, in_=wqT[:, :].rearrange("(e p) d -> p e d", p=P))
                nc.sync.dma_start(out=wk_sb, in_=wkT[:, :].rearrange("(e p) d -> p e d", p=P))
                nc.sync.dma_start(out=wv_sb, in_=wvT[:, :].rearrange("(e p) d -> p e d", p=P))

                xp = xpool.tile([P, EC, T], BF16)
                for e in range(EC):
                    nc.sync.dma_start(out=xp[:, e, :], in_=xT[ts(e, P), :])

                # Q^T, K^T : [dloc (part), T]
                for dst, wsb in ((qT, wq_sb), (kT, wk_sb)):
                    for dc in range(DC):
                        for w in range(T // TW):
                            ps = psq.tile([P, TW], F32, tag="qa", name="ps_qk")
                            for e in range(EC):
                                nc.tensor.matmul(
                                    ps,
                                    wsb[:, e, ts(dc, P)],
                                    xp[:, e, ds(TW * w, TW)],
                                    start=(e == 0),
                                    stop=(e == EC - 1),
                                )
                            nc.vector.tensor_copy(dst[:, dc, ds(TW * w, TW)], ps)

                # V natural : [t (part), d]
                for t in range(TC):
                    psv = psq.tile([P, HPC * DK], F32, tag="qa", name="ps_v")
                    for e in range(EC):
                        nc.tensor.matmul(
                            psv,
                            xp[:, e, ts(t, P)],
                            wv_sb[:, e, :],
                            start=(e == 0),
                            stop=(e == EC - 1),
                        )
                    nc.vector.tensor_copy(
                        vsb.rearrange("p t (h c) -> p t h c", c=DK + 1)[:, t, :, 0:DK],
                        psv.rearrange("p (h d) -> p h d", d=DK),
                    )

            # ---------------- attention ----------------
            with tc.tile_pool(name="att", bufs=1) as att_pool:
                att2 = []
                for h in range(HPC):
                    a = att_pool.tile([P, T], BF16, name=f"att2_{h}", tag=f"att2_{h}")
                    att2.append(a)
                    # last col of shifted half never written; keep sim happy
                    nc.vector.memset(a[DK : 2 * DK, T - 1 : T], 0.0)

                with (
                    tc.tile_pool(name="exps", bufs=8) as epool,
                    tc.tile_pool(name="rec", bufs=2) as rpool,
                    tc.tile_pool(name="psa", bufs=1, space="PSUM") as psa,
                ):
                    WARMERS = 3  # zero-matmuls per jc to keep the PE HAM-warm
                    for w in range(W):
                        pvs = [
                            psa.tile([P, TW], F32, tag=f"pv{h}", bufs=1, name=f"pv{h}")
                            for h in range(HPC)
                        ]
                        njc = JPW * (w + 1)
                        pend = None
                        for jc in range(njc):
                            qq = jc - JPW * w  # >=0 on causal-diagonal blocks
                            ess = []
                            for p in range(2):
                                st = psa.tile([P, 2 * TW], F32, tag="s", bufs=2, name="st")
                                for sub in range(2):
                                    nc.tensor.matmul(
                                        st[:, ds(TW * sub, TW)],
                                        kT[ds(DK * sub, DK), p, ts(jc, P)],
                                        qT[ds(DK * sub, DK), p, ds(TW * w, TW)],
                                        start=True,
                                        stop=True,
                                    )
                                es = epool.tile([P, 2 * TW], BF16, name="es")
                                nc.scalar.activation(es, st, AF.Exp, scale=1.0 / 8.0)
                                if qq >= 0:
                                    for sub in range(2):
                                        nc.gpsimd.affine_select(
                                            out=es[:, ds(TW * sub, TW)],
                                            in_=es[:, ds(TW * sub, TW)],
                                            pattern=[[1, TW]],
                                            compare_op=mybir.AluOpType.is_ge,
                                            fill=0.0,
                                            base=-P * qq,
                                            channel_multiplier=-1,
                                        )
                                ess.append(es)
                            if pend is not None:
                                _emit_pv(nc, pvs, vsb, zer, wsrc, pend[0], pend[1], njc, WARMERS)
                            pend = (ess, jc)
                        _emit_pv(nc, pvs, vsb, zer, wsrc, pend[0], pend[1], njc, 0)

                        # ---- normalization (per window) ----
                        # gather the 4 denominator rows, spread across partitions via
                        # DMA, one cheap 16-elem/lane reciprocal, spread back as bf16
                        dns = rpool.tile([P, HPC * TW], F32, name="dns", tag="dns")
                        praws = []
                        for h in range(HPC):
                            praw = rpool.tile([P, TW], BF16, name="praw", tag=f"praw{h}", bufs=2)
                            nc.vector.tensor_copy(praw[0:DK, :], pvs[h][0:DK, :])
                            nc.vector.tensor_copy(
                                dns[DK : DK + 1, ds(TW * h, TW)], pvs[h][DK : DK + 1, :]
                            )
                            praws.append(praw)
                        NSP = HPC * TW // P  # 16 elems per lane
                        sp = rpool.tile([P, 2 * NSP], F32, name="sp", tag="sp")
                        nc.sync.dma_start(
                            out=sp[:, 0:NSP],
                            in_=dns[DK : DK + 1, :].rearrange("a (p c) -> a p c", c=NSP),
                        )
                        nc.vector.reciprocal(out=sp[:, NSP : 2 * NSP], in_=sp[:, 0:NSP])
                        spb = rpool.tile([P, 2 * NSP], BF16, name="spb", tag="spb")
                        nc.vector.tensor_copy(spb[:, 0:NSP], sp[:, NSP : 2 * NSP])
                        recb = rpool.tile([P, HPC * TW], BF16, name="recb", tag="recb")
                        nc.sync.dma_start(
                            out=recb[DK : DK + 1, :].rearrange("a (p c) -> a p c", c=NSP),
                            in_=spb[:, 0:NSP],
                        )
                        for h in range(HPC):
                            rt = psa.tile([P, 2 * TW], F32, tag="s", bufs=2, name="rt")
                            nc.tensor.matmul(
                                rt[0:DK, 0:TW],
                                ones[DK : DK + 1, 0:DK],
                                recb[DK : DK + 1, ds(TW * h, TW)],
                                start=True,
                                stop=True,
                            )
                            nc.vector.tensor_mul(
                                att2[h][0:DK, ds(TW * w, TW)],
                                rt[0:DK, 0:TW],
                                praws[h][0:DK, :],
                            )
                            if w == 0:
                                nc.sync.dma_start(
                                    out=att2[h][DK : 2 * DK, 0 : TW - 1],
                                    in_=att2[h][0:DK, 1:TW],
                                )
                            else:
                                nc.sync.dma_start(
                                    out=att2[h][DK : 2 * DK, TW * w - 1 : TW * (w + 1) - 1],
                                    in_=att2[h][0:DK, ds(TW * w, TW)],
                                )

                # ---------------- output projection ----------------
                with (
                    tc.tile_pool(name="yout", bufs=2) as ypool,
                    tc.tile_pool(name="psy", bufs=2, space="PSUM") as psy,
                ):
                    for h in range(HPC):
                        a2v = att2[h].rearrange("p (r t) -> p r t", t=TT)  # [P, RR, TT]
                        for fw in range(FW):
                            yp = psy.tile([P, 512], F32, tag="y", name="yp")
                            for m in range(EC):
                                nc.tensor.matmul(
                                    yp[0:RR, :],
                                    a2v[:, :, 2 * m : 2 * m + 1],
                                    wp_sb[:, m, ds(512 * fw, 512)],
                                    start=(m == 0),
                                    stop=False,
                                )
                            nc.tensor.matmul(
                                yp[0:RR, :],
                                ones[0:1, 0:RR],
                                bp_sb[0:1, ds(512 * fw, 512)],
                                start=False,
                                stop=True,
                            )
                            ysb = ypool.tile([P, 512], F32, name="ysb")
                            nc.vector.tensor_copy(ysb[0:RR, :], yp[0:RR, :])
                            nc.sync.dma_start(
                                out=y[ds(RR * h, RR), ds(512 * fw, 512)], in_=ysb[0:RR, :]
                            )
    nc.compile()
    return nc


def _emit_pv(nc, pvs, vsb, zer, wsrc, ess, jc, njc, warmers):
    """P@V matmuls (M=65: V plus ones column -> denominator row) for one jc,
    then `warmers` zero-matmuls that accumulate 0 into an open pv group —
    pure PE-activity filler so the HAM clock gate stays at full speed."""
    for p in range(2):
        es = ess[p]
        for sub in range(2):
            h = 2 * p + sub
            nc.tensor.matmul(
                pvs[h][0 : DK + 1, :],
                vsb[:, jc, ds((DK + 1) * h, DK + 1)],
                es[:, ds(TW * sub, TW)],
                start=(jc == 0),
                stop=(jc == njc - 1),
            )
    if jc < njc - 1:
        for k in range(warmers):
            nc.tensor.matmul(
                pvs[(jc + k) % HPC][0 : DK + 1, :],
                zer,
                wsrc,
                start=False,
                stop=False,
            )



_CACHE = {}
LAST_RESULT = None


def _get_nc(T=2048):
    key = ("nc", T)
    if key not in _CACHE:
        _CACHE[key] = build_nc(T=T)
    return _CACHE[key]


def make_in_maps(x, w_qkv, w_proj, b_proj):
    B, T, _E = x.shape
    in_maps = []
    wpTh = np.ascontiguousarray(w_proj.T.astype(BF16NP))
    bph = np.ascontiguousarray(b_proj.reshape(1, E).astype(BF16NP))
    xTs = [np.ascontiguousarray(x[b].T.astype(BF16NP)) for b in range(B)]
    for c in range(8):
        b, g = divmod(c, 4)
        r0 = HPC * DK * g  # 256*g
        sl = slice(r0, r0 + HPC * DK)
        in_maps.append(
            {
                "xT": xTs[b],
                "wqT": np.ascontiguousarray(w_qkv[sl, :].T.astype(BF16NP)),
                "wkT": np.ascontiguousarray(w_qkv[E:][sl, :].T.astype(BF16NP)),
                "wvT": np.ascontiguousarray(w_qkv[2 * E :][sl, :].T.astype(BF16NP)),
                "wpT": wpTh,
                "bp": bph,
            }
        )
    return in_maps


def kernel(x, w_qkv, w_proj, b_proj):
    global LAST_RESULT
    from concourse.bass_utils import run_bass_kernel_spmd

    x = np.asarray(x, dtype=np.float32)
    w_qkv = np.asarray(w_qkv, dtype=np.float32)
    w_proj = np.asarray(w_proj, dtype=np.float32)
    b_proj = np.asarray(b_proj, dtype=np.float32)
    B, T, _E = x.shape

    nc = _get_nc(T=T)
    in_maps = make_in_maps(x, w_qkv, w_proj, b_proj)
    res = run_bass_kernel_spmd(nc, in_maps, core_ids=list(range(8)))
    LAST_RESULT = res

    out = np.empty((B, T, E), dtype=np.float32)
    rows = HPC * ((T * DK) // E)  # 512 rows per core
    for c in range(8):
        b, g = divmod(c, 4)
        out[b, rows * g : rows * (g + 1), :] = res.results[c]["y"]
    return out


# revision 14
# speedup vs baseline: 1.5335x; 1.0535x over previous
"""Causal attention (with faithful missing-head-transpose reshape bug) on 8 Trainium2 cores.

Problem: B=2, T=2048, E=1024, H=16, dk=64.
  qkv = x @ w_qkv.T ; q,k,v split; per-head causal softmax attention;
  out = att_out[B,H,T,dk].reshape(B,T,E)  (NO head transpose — faithful bug);
  y = out @ w_proj.T + b_proj

Key observation: because of the missing transpose, output rows
y[b, 128h : 128h+128, :] depend ONLY on head h.  Sharding (batch x head-group)
over 8 cores therefore needs NO collectives: core c handles batch c//4 and
heads 4*(c%4) .. 4*(c%4)+3, producing output rows [512g, 512g+512) of batch b.

Per-core kernel (bf16 matmuls, fp32 PSUM accumulation, fp32 softmax math):
  - host supplies x^T [E,T] and pre-transposed weight slices in bf16
  - Q^T,K^T [256,T] and V [T,256] via matmuls
  - scores computed TRANSPOSED: S^T[j,i] (keys on partitions) so that
    exp(S^T) is directly the lhsT-ready P^T for the P@V matmul
  - V is stored with a ones-column per head: the P@V matmul's row 64 is then
    the softmax denominator for free
  - causal masking via gpsimd.affine_select on the exp tiles (diag blocks only;
    fully-masked blocks are skipped entirely)
  - normalization: fast-approx reciprocal of the denom row, broadcast across
    partitions with a K=1 outer-product matmul, one DVE multiply
  - the buggy reshape is free in row-major DRAM; the projection's lhsT
    (R^T chunks) is read from a [dup, shift-by-one] att2 buffer with a
    stride-16 access pattern
"""

import os
import sys

import numpy as np

for _p in ("/opt/trn_rl_repo", "/root/.axon_site/_ro/trn_rl_repo"):
    if os.path.isdir(_p) and _p not in sys.path:
        sys.path.insert(0, _p)

import ml_dtypes  # noqa: E402

import concourse.bacc as bacc  # noqa: E402
import concourse.mybir as mybir  # noqa: E402
from concourse.bass import ds, ts  # noqa: E402
from concourse.tile import TileContext  # noqa: E402

F32 = mybir.dt.float32
BF16 = mybir.dt.bfloat16
AF = mybir.ActivationFunctionType
BF16NP = ml_dtypes.bfloat16

P = 128
E = 1024
DK = 64
HPC = 4  # heads per core
TW = 512  # i-window for scores / pv matmuls
EC = E // P  # 8 e-chunks
DC = (HPC * DK) // P  # 2 chunks of per-core qk features
FW = E // 512  # 2 output-feature windows


def build_nc(T=2048):
    W = T // TW  # i-windows
    JPW = TW // P  # j-chunks per window (4)
    TC = T // P  # t-chunks for V
    RR = (T * DK) // E  # rows of R per head (T/16)
    TT = E // DK  # 16 t-positions per R row

    nc = bacc.Bacc("TRN2", target_bir_lowering=False, debug=False)
    xT = nc.declare_dram_parameter("xT", [E, T], BF16, isOutput=False)
    wqT = nc.declare_dram_parameter("wqT", [E, HPC * DK], BF16, isOutput=False)
    wkT = nc.declare_dram_parameter("wkT", [E, HPC * DK], BF16, isOutput=False)
    wvT = nc.declare_dram_parameter("wvT", [E, HPC * DK], BF16, isOutput=False)
    wpT = nc.declare_dram_parameter("wpT", [E, E], BF16, isOutput=False)
    bp = nc.declare_dram_parameter("bp", [1, E], BF16, isOutput=False)
    y = nc.declare_dram_parameter("y", [HPC * RR, E], F32, isOutput=True)

    with nc.allow_low_precision(reason="bf16 matmuls; accumulation stays fp32 in PSUM"), TileContext(nc) as tc:
        with (
            tc.tile_pool(name="const", bufs=1) as const,
            tc.tile_pool(name="qkvout", bufs=1) as qkv_pool,
            tc.tile_pool(name="wp", bufs=1) as wp_pool,
        ):
            ones = const.tile([P, P], BF16)
            nc.vector.memset(ones, 1.0)
            bp_sb = const.tile([1, E], BF16)
            nc.sync.dma_start(out=bp_sb, in_=bp[:, :])

            wp_sb = wp_pool.tile([P, EC, E], BF16)
            nc.sync.dma_start(out=wp_sb, in_=wpT[:, :].rearrange("(e p) f -> p e f", p=P))

            qT = qkv_pool.tile([P, DC, T], BF16)
            kT = qkv_pool.tile([P, DC, T], BF16)
            vsb = qkv_pool.tile([P, TC, HPC * (DK + 1)], BF16)
            # ones column per head (row 64 of each P@V psum = softmax denominator)
            nc.vector.memset(
                vsb.rearrange("p t (h c) -> p t h c", c=DK + 1)[:, :, :, DK : DK + 1], 1.0
            )
            zer = const.tile([P, DK + 1], BF16)
            nc.vector.memset(zer, 0.0)
            wsrc = const.tile([P, TW], BF16)
            nc.vector.memset(wsrc, 0.0)
            # causal masks for the 4 diagonal-block offsets: keep j <= i - 128*q
            masks = []
            for q in range(JPW):
                mk = const.tile([P, TW], BF16, name=f"mask{q}", tag=f"mask{q}")
                nc.vector.memset(mk, 1.0)
                nc.gpsimd.affine_select(
                    out=mk,
                    in_=mk,
                    pattern=[[1, TW]],
                    compare_op=mybir.AluOpType.is_ge,
                    fill=0.0,
                    base=-P * q,
                    channel_multiplier=-1,
                )
                masks.append(mk)

            # ---------------- QKV projections ----------------
            with (
                tc.tile_pool(name="xin", bufs=1) as xpool,
                tc.tile_pool(name="wqkv", bufs=1) as wq_pool,
                tc.tile_pool(name="psq", bufs=4, space="PSUM") as psq,
            ):
                wq_sb = wq_pool.tile([P, EC, HPC * DK], BF16)
                wk_sb = wq_pool.tile([P, EC, HPC * DK], BF16)
                wv_sb = wq_pool.tile([P, EC, HPC * DK], BF16)
                nc.sync.dma_start(out=wq_sb, in_=wqT[:, :].rearrange("(e p) d -> p e d", p=P))
                nc.sync.dma_start(out=wk_sb, in_=wkT[:, :].rearrange("(e p) d -> p e d", p=P))
                nc.sync.dma_start(out=wv_sb, in_=wvT[:, :].rearrange("(e p) d -> p e d", p=P))

                xp = xpool.tile([P, EC, T], BF16)
                for e in range(EC):
                    nc.sync.dma_start(out=xp[:, e, :], in_=xT[ts(e, P), :])

                # Q^T, K^T : [dloc (part), T]
                for dst, wsb in ((qT, wq_sb), (kT, wk_sb)):
                    for dc in range(DC):
                        for w in range(T // TW):
                            ps = psq.tile([P, TW], F32, tag="qa", name="ps_qk")
                            for e in range(EC):
                                nc.tensor.matmul(
                                    ps,
                                    wsb[:, e, ts(dc, P)],
                                    xp[:, e, ds(TW * w, TW)],
                                    start=(e == 0),
                                    stop=(e == EC - 1),
                                )
                            nc.vector.tensor_copy(dst[:, dc, ds(TW * w, TW)], ps)

                # V natural : [t (part), d]
                for t in range(TC):
                    psv = psq.tile([P, HPC * DK], F32, tag="qa", name="ps_v")
                    for e in range(EC):
                        nc.tensor.matmul(
                            psv,
                            xp[:, e, ts(t, P)],
                            wv_sb[:, e, :],
                            start=(e == 0),
                            stop=(e == EC - 1),
                        )
                    nc.vector.tensor_copy(
                        vsb.rearrange("p t (h c) -> p t h c", c=DK + 1)[:, t, :, 0:DK],
                        psv.rearrange("p (h d) -> p h d", d=DK),
                    )

            # ---------------- attention ----------------
            with tc.tile_pool(name="att", bufs=1) as att_pool:
                att2 = []
                for h in range(HPC):
                    a = att_pool.tile([P, T], BF16, name=f"att2_{h}", tag=f"att2_{h}")
                    att2.append(a)
                    # last col of shifted half never written; keep sim happy
                    nc.vector.memset(a[DK : 2 * DK, T - 1 : T], 0.0)

                with (
                    tc.tile_pool(name="exps", bufs=8) as epool,
                    tc.tile_pool(name="rec", bufs=2) as rpool,
                    tc.tile_pool(name="psa", bufs=1, space="PSUM") as psa,
                ):
                    WARMERS = 3  # zero-matmuls per jc to keep the PE HAM-warm
                    NSP = HPC * TW // P  # denom elems per lane after spread
                    norm2 = None
                    for w in range(W):
                        pvs = [
                            psa.tile([P, TW], F32, tag=f"pv{h}", bufs=1, name=f"pv{h}")
                            for h in range(HPC)
                        ]
                        njc = JPW * (w + 1)
                        pend = None
                        for jc in range(njc):
                            qq = jc - JPW * w  # >=0 on causal-diagonal blocks
                            ess = []
                            for p in range(2):
                                st = psa.tile([P, 2 * TW], F32, tag="s", bufs=2, name="st")
                                for sub in range(2):
                                    nc.tensor.matmul(
                                        st[:, ds(TW * sub, TW)],
                                        kT[ds(DK * sub, DK), p, ts(jc, P)],
                                        qT[ds(DK * sub, DK), p, ds(TW * w, TW)],
                                        start=True,
                                        stop=True,
                                    )
                                es = epool.tile([P, 2 * TW], BF16, name="es")
                                nc.scalar.activation(es, st, AF.Exp, scale=1.0 / 8.0)
                                if qq >= 0:
                                    for sub in range(2):
                                        nc.vector.tensor_mul(
                                            es[:, ds(TW * sub, TW)],
                                            es[:, ds(TW * sub, TW)],
                                            masks[qq],
                                        )
                                ess.append(es)
                            if pend is not None:
                                _emit_pv(nc, pvs, vsb, zer, wsrc, pend[0], pend[1], njc, WARMERS)
                            pend = (ess, jc)
                            if jc == 1 and norm2 is not None:
                                norm2()
                                norm2 = None
                        _emit_pv(nc, pvs, vsb, zer, wsrc, pend[0], pend[1], njc, 0)

                        # ---- normalization part 1: drain pv banks ----
                        dns = rpool.tile([P, HPC * TW], F32, name="dns", tag="dns")
                        praws = []
                        for h in range(HPC):
                            praw = rpool.tile([P, TW], BF16, name="praw", tag=f"praw{h}", bufs=2)
                            nc.vector.tensor_copy(praw[0:DK, :], pvs[h][0:DK, :])
                            nc.vector.tensor_copy(
                                dns[DK : DK + 1, ds(TW * h, TW)], pvs[h][DK : DK + 1, :]
                            )
                            praws.append(praw)

                        def _norm2(w=w, dns=dns, praws=praws):
                            # spread denom row across lanes, reciprocal, spread back
                            sp = rpool.tile([P, 2 * NSP], F32, name="sp", tag="sp")
                            nc.sync.dma_start(
                                out=sp[:, 0:NSP],
                                in_=dns[DK : DK + 1, :].rearrange("a (p c) -> a p c", c=NSP),
                            )
                            nc.vector.reciprocal(out=sp[:, NSP : 2 * NSP], in_=sp[:, 0:NSP])
                            spb = rpool.tile([P, 2 * NSP], BF16, name="spb", tag="spb")
                            nc.vector.tensor_copy(spb[:, 0:NSP], sp[:, NSP : 2 * NSP])
                            recb = rpool.tile([P, HPC * TW], BF16, name="recb", tag="recb")
                            nc.sync.dma_start(
                                out=recb[DK : DK + 1, :].rearrange("a (p c) -> a p c", c=NSP),
                                in_=spb[:, 0:NSP],
                            )
                            for h in range(HPC):
                                rt = psa.tile([P, 2 * TW], F32, tag="s", bufs=2, name="rt")
                                nc.tensor.matmul(
                                    rt[0:DK, 0:TW],
                                    ones[DK : DK + 1, 0:DK],
                                    recb[DK : DK + 1, ds(TW * h, TW)],
                                    start=True,
                                    stop=True,
                                )
                                nc.vector.tensor_mul(
                                    att2[h][0:DK, ds(TW * w, TW)],
                                    rt[0:DK, 0:TW],
                                    praws[h][0:DK, :],
                                )
                                if w == 0:
                                    nc.sync.dma_start(
                                        out=att2[h][DK : 2 * DK, 0 : TW - 1],
                                        in_=att2[h][0:DK, 1:TW],
                                    )
                                else:
                                    nc.sync.dma_start(
                                        out=att2[h][DK : 2 * DK, TW * w - 1 : TW * (w + 1) - 1],
                                        in_=att2[h][0:DK, ds(TW * w, TW)],
                                    )

                        norm2 = _norm2
                    norm2()

                # ---------------- output projection ----------------
                with (
                    tc.tile_pool(name="yout", bufs=2) as ypool,
                    tc.tile_pool(name="psy", bufs=2, space="PSUM") as psy,
                ):
                    for h in range(HPC):
                        a2v = att2[h].rearrange("p (r t) -> p r t", t=TT)  # [P, RR, TT]
                        for fw in range(FW):
                            yp = psy.tile([P, 512], F32, tag="y", name="yp")
                            for m in range(EC):
                                nc.tensor.matmul(
                                    yp[0:RR, :],
                                    a2v[:, :, 2 * m : 2 * m + 1],
                                    wp_sb[:, m, ds(512 * fw, 512)],
                                    start=(m == 0),
                                    stop=False,
                                )
                            nc.tensor.matmul(
                                yp[0:RR, :],
                                ones[0:1, 0:RR],
                                bp_sb[0:1, ds(512 * fw, 512)],
                                start=False,
                                stop=True,
                            )
                            ysb = ypool.tile([P, 512], F32, name="ysb")
                            nc.vector.tensor_copy(ysb[0:RR, :], yp[0:RR, :])
                            nc.sync.dma_start(
                                out=y[ds(RR * h, RR), ds(512 * fw, 512)], in_=ysb[0:RR, :]
                            )
    nc.compile()
    return nc


def _emit_pv(nc, pvs, vsb, zer, wsrc, ess, jc, njc, warmers):
    """P@V matmuls (M=65: V plus ones column -> denominator row) for one jc,
    then `warmers` zero-matmuls that accumulate 0 into an open pv group —
    pure PE-activity filler so the HAM clock gate stays at full speed."""
    for p in range(2):
        es = ess[p]
        for sub in range(2):
            h = 2 * p + sub
            nc.tensor.matmul(
                pvs[h][0 : DK + 1, :],
                vsb[:, jc, ds((DK + 1) * h, DK + 1)],
                es[:, ds(TW * sub, TW)],
                start=(jc == 0),
                stop=(jc == njc - 1),
            )
    if jc < njc - 1:
        for k in range(warmers):
            nc.tensor.matmul(
                pvs[(jc + k) % HPC][0 : DK + 1, :],
                zer,
                wsrc,
                start=False,
                stop=False,
            )



_CACHE = {}
LAST_RESULT = None


def _get_nc(T=2048):
    key = ("nc", T)
    if key not in _CACHE:
        _CACHE[key] = build_nc(T=T)
    return _CACHE[key]


def make_in_maps(x, w_qkv, w_proj, b_proj):
    B, T, _E = x.shape
    in_maps = []
    wpTh = np.ascontiguousarray(w_proj.T.astype(BF16NP))
    bph = np.ascontiguousarray(b_proj.reshape(1, E).astype(BF16NP))
    xTs = [np.ascontiguousarray(x[b].T.astype(BF16NP)) for b in range(B)]
    for c in range(8):
        b, g = divmod(c, 4)
        r0 = HPC * DK * g  # 256*g
        sl = slice(r0, r0 + HPC * DK)
        in_maps.append(
            {
                "xT": xTs[b],
                "wqT": np.ascontiguousarray(w_qkv[sl, :].T.astype(BF16NP)),
                "wkT": np.ascontiguousarray(w_qkv[E:][sl, :].T.astype(BF16NP)),
                "wvT": np.ascontiguousarray(w_qkv[2 * E :][sl, :].T.astype(BF16NP)),
                "wpT": wpTh,
                "bp": bph,
            }
        )
    return in_maps


def kernel(x, w_qkv, w_proj, b_proj):
    global LAST_RESULT
    from concourse.bass_utils import run_bass_kernel_spmd

    x = np.asarray(x, dtype=np.float32)
    w_qkv = np.asarray(w_qkv, dtype=np.float32)
    w_proj = np.asarray(w_proj, dtype=np.float32)
    b_proj = np.asarray(b_proj, dtype=np.float32)
    B, T, _E = x.shape

    nc = _get_nc(T=T)
    in_maps = make_in_maps(x, w_qkv, w_proj, b_proj)
    res = run_bass_kernel_spmd(nc, in_maps, core_ids=list(range(8)))
    LAST_RESULT = res

    out = np.empty((B, T, E), dtype=np.float32)
    rows = HPC * ((T * DK) // E)  # 512 rows per core
    for c in range(8):
        b, g = divmod(c, 4)
        out[b, rows * g : rows * (g + 1), :] = res.results[c]["y"]
    return out
